# revision 4
# baseline (speedup 1.0000x reference)
"""Localized embedding layer (separable 5x5 Gaussian stencil) on 8 trn2 cores.

Math: out[i,j,:] = sum_{|di|<=2,|dj|<=2} w(di)w(dj) H[i+di,j+dj,:] / den(i,j)
with w(d) = exp(-c*d^2), c = TILE^2/(2 sigma^2), den(i,j) = r(i)*r(j) rank-1.

Per core (32 output grid rows + 2-row halo each side, zero padded):
  - i-conv (across grid rows)  -> DVE/GPSIMD: 4 fused ops per [128,512] tile
  - j-conv (across partitions) -> TensorE: banded 256x256 matrix matmul,
    1/(r(j)*W_full) folded into the matrix, accumulated over 2 K-chunks in PSUM
  - ScalarE: PSUM->SBUF copy with per-row scale W_full/r(i) (=1 in interior)
  - DMA out
"""

import sys
import numpy as np

if "/opt/trn_rl_repo" not in sys.path:
    sys.path.insert(0, "/opt/trn_rl_repo")

G = 256          # grid side
D = 512          # feature dim
P = 2            # grid_step halo
NC = 8           # cores
RPC = G // NC    # rows per core = 32
TILE = 448.0
SIGMA = 200.0

_cache = {}


def _weights():
    c = TILE * TILE / (2.0 * SIGMA * SIGMA)
    w = np.exp(-c * np.arange(-P, P + 1) ** 2)          # [w2,w1,1,w1,w2] f64
    return w


def _r_vec():
    """r(i) = sum of valid 1D taps at row i (same for columns)."""
    w = _weights()
    r = np.zeros(G)
    for d in range(-P, P + 1):
        lo, hi = max(0, -d), min(G, G - d)
        r[lo:hi] += w[d + P]
    return r


def _host_consts():
    """Build lhsT chunks for the j-conv matmul and per-core row scales."""
    w = _weights()
    r = _r_vec()
    w_full = w.sum()
    # Banded matrix Bp[jout, jin] = w(jout-jin) / (r(jout) * w_full)
    Bp = np.zeros((G, G))
    for d in range(-P, P + 1):
        for jout in range(G):
            jin = jout + d
            if 0 <= jin < G:
                Bp[jout, jin] = w[d + P] / (r[jout] * w_full)
    # lhsT layout [k, hk, hm, m] with lhsT[k,hk,hm,m] = Bp[128*hm+m, 128*hk+k]
    wmat = np.zeros((128, 2, 2, 128), dtype=np.float32)
    for hk in range(2):
        for hm in range(2):
            blk = Bp[128 * hm:128 * hm + 128, 128 * hk:128 * hk + 128]
            wmat[:, hk, hm, :] = blk.T.astype(np.float32)
    # per-core scale: [128, RPC] column i = w_full / r(32c + i), broadcast 128x
    scales = []
    for c in range(NC):
        s = (w_full / r[RPC * c: RPC * (c + 1)]).astype(np.float32)
        scales.append(np.broadcast_to(s[None, :], (128, RPC)).copy())
    return wmat, scales


def _build_nc():
    import concourse.bass as bass
    import concourse.mybir as mybir
    import concourse.tile as tile
    from concourse import bacc

    f32 = mybir.dt.float32
    add = mybir.AluOpType.add
    mult = mybir.AluOpType.mult

    w = _weights()
    w1 = float(w[1])
    w2 = float(w[0])

    nc = bacc.Bacc(None, target_bir_lowering=False, debug=False)
    x_dram = nc.declare_dram_parameter("x", [RPC + 2 * P, G, D], f32, isOutput=False)
    wm_dram = nc.declare_dram_parameter("wmat", [128, 2, 2, 128], f32, isOutput=False)
    sc_dram = nc.declare_dram_parameter("scale", [128, RPC], f32, isOutput=False)
    y_dram = nc.declare_dram_parameter("y", [RPC, G, D], f32, isOutput=True)

    with tile.TileContext(nc) as tc:
        with (
            tc.tile_pool(name="const", bufs=1) as cpool,
            tc.tile_pool(name="x", bufs=9) as xpool,
            tc.tile_pool(name="tmp", bufs=8) as tpool,
            tc.tile_pool(name="tacc", bufs=6) as tapool,
            tc.tile_pool(name="out", bufs=6) as opool,
            tc.tile_pool(name="psum", bufs=4, space="PSUM") as ppool,
        ):
            wt = cpool.tile([128, 2, 2, 128], f32)
            nc.sync.dma_start(wt[:], wm_dram[:])
            st = cpool.tile([128, RPC], f32)
            nc.sync.dma_start(st[:], sc_dram[:])

            xt = {}

            def load_row(r):
                t = xpool.tile([128, 2, D], f32, tag="xrow")
                nc.sync.dma_start(
                    t[:], x_dram[r].rearrange("(h p) d -> p h d", p=128)
                )
                xt[r] = t

            for r in range(5):
                load_row(r)

            for i in range(RPC):
                if i > 0:
                    load_row(i + 4)
                tacc = []
                for h in range(2):
                    a0 = xt[i][:, h, :]
                    a1 = xt[i + 1][:, h, :]
                    a2 = xt[i + 2][:, h, :]
                    a3 = xt[i + 3][:, h, :]
                    a4 = xt[i + 4][:, h, :]
                    # engines split: gpsimd takes one TT add, vector the rest
                    t1 = tpool.tile([128, D], f32, tag="t1")
                    nc.vector.tensor_tensor(t1[:], a1, a3, add)
                    t2 = tpool.tile([128, D], f32, tag="t2")
                    nc.gpsimd.tensor_tensor(t2[:], a0, a4, add)
                    t3 = tpool.tile([128, D], f32, tag="t3")
                    nc.vector.scalar_tensor_tensor(t3[:], t2[:], w2 / w1, t1[:], mult, add)
                    tt = tapool.tile([128, D], f32, tag="tacc")
                    nc.vector.scalar_tensor_tensor(tt[:], t3[:], w1, a2, mult, add)
                    tacc.append(tt)
                for hm in range(2):
                    ps = ppool.tile([128, D], f32, tag="ps")
                    nc.tensor.matmul(
                        ps[:], wt[:, 0, hm, :], tacc[0][:], start=True, stop=False
                    )
                    nc.tensor.matmul(
                        ps[:], wt[:, 1, hm, :], tacc[1][:], start=False, stop=True
                    )
                    ob = opool.tile([128, D], f32, tag="ob")
                    nc.scalar.mul(ob[:], ps[:], st[:, i:i + 1])
                    nc.sync.dma_start(
                        y_dram[i, 128 * hm:128 * hm + 128, :], ob[:]
                    )
    nc.finalize()
    return nc


def _get_program():
    if "nc" not in _cache:
        _cache["nc"] = _build_nc()
        _cache["consts"] = _host_consts()
    return _cache["nc"], _cache["consts"]


def kernel(H, xy=None):
    from concourse.bass_utils import run_bass_kernel_spmd

    nc, (wmat, scales) = _get_program()
    H3 = np.ascontiguousarray(H.reshape(G, G, D).astype(np.float32))
    Hp = np.zeros((G + 2 * P, G, D), dtype=np.float32)
    Hp[P:P + G] = H3
    in_maps = []
    for c in range(NC):
        shard = np.ascontiguousarray(Hp[RPC * c: RPC * c + RPC + 2 * P])
        in_maps.append({"x": shard, "wmat": wmat, "scale": scales[c]})
    res = run_bass_kernel_spmd(nc, in_maps, list(range(NC))).results
    out = np.concatenate([res[c]["y"].reshape(RPC * G, D) for c in range(NC)], axis=0)
    return out


# revision 8
# speedup vs baseline: 12.3368x; 12.3368x over previous
"""Localized embedding layer (separable 5x5 Gaussian stencil) on 8 trn2 cores.

Math: out[i,j,:] = sum_{|di|<=2,|dj|<=2} w(di)w(dj) H[i+di,j+dj,:] / den(i,j)
with w(d) = exp(-c*d^2), c = TILE^2/(2 sigma^2), den(i,j) = r(i)*r(j) rank-1.

Per core (32 output grid rows + 2-row halo each side, zero padded):
  - i-conv (across grid rows)  -> DVE/GPSIMD: 4 fused ops per [128,512] tile
  - j-conv (across partitions) -> TensorE: banded 256x256 matrix matmul,
    1/(r(j)*W_full) folded into the matrix, accumulated over 2 K-chunks in PSUM
  - ScalarE: PSUM->SBUF copy with per-row scale W_full/r(i) (=1 in interior)
  - DMA out
"""

import sys
import numpy as np

if "/opt/trn_rl_repo" not in sys.path:
    sys.path.insert(0, "/opt/trn_rl_repo")

G = 256          # grid side
D = 512          # feature dim
P = 2            # grid_step halo
NC = 8           # cores
RPC = G // NC    # rows per core = 32
TILE = 448.0
SIGMA = 200.0

_cache = {}


def _weights():
    c = TILE * TILE / (2.0 * SIGMA * SIGMA)
    w = np.exp(-c * np.arange(-P, P + 1) ** 2)          # [w2,w1,1,w1,w2] f64
    return w


def _r_vec():
    """r(i) = sum of valid 1D taps at row i (same for columns)."""
    w = _weights()
    r = np.zeros(G)
    for d in range(-P, P + 1):
        lo, hi = max(0, -d), min(G, G - d)
        r[lo:hi] += w[d + P]
    return r


def _host_consts():
    """Build lhsT chunks for the j-conv matmul and per-core row scales."""
    w = _weights()
    r = _r_vec()
    w_full = w.sum()
    # Banded matrix Bp[jout, jin] = w(jout-jin) / (r(jout) * w_full)
    Bp = np.zeros((G, G))
    for d in range(-P, P + 1):
        for jout in range(G):
            jin = jout + d
            if 0 <= jin < G:
                Bp[jout, jin] = w[d + P] / (r[jout] * w_full)
    # lhsT layout [k, hk, hm, m] with lhsT[k,hk,hm,m] = Bp[128*hm+m, 128*hk+k]
    wmat = np.zeros((128, 2, 2, 128), dtype=np.float32)
    for hk in range(2):
        for hm in range(2):
            blk = Bp[128 * hm:128 * hm + 128, 128 * hk:128 * hk + 128]
            wmat[:, hk, hm, :] = blk.T.astype(np.float32)
    # per-core scale: [128, RPC] column i = w_full / r(32c + i), broadcast 128x
    scales = []
    for c in range(NC):
        s = (w_full / r[RPC * c: RPC * (c + 1)]).astype(np.float32)
        scales.append(np.broadcast_to(s[None, :], (128, RPC)).copy())
    return wmat, scales


def _build_nc(repeats=1):
    import concourse.bass as bass
    import concourse.mybir as mybir
    import concourse.tile as tile
    from concourse import bacc

    f32 = mybir.dt.float32
    add = mybir.AluOpType.add
    mult = mybir.AluOpType.mult

    w = _weights()
    w1 = float(w[1])
    w2 = float(w[0])

    nc = bacc.Bacc(None, target_bir_lowering=False, debug=False)
    x_dram = nc.declare_dram_parameter("x", [RPC + 2 * P, G, D], f32, isOutput=False)
    wm_dram = nc.declare_dram_parameter("wmat", [128, 2, 2, 128], f32, isOutput=False)
    sc_dram = nc.declare_dram_parameter("scale", [128, RPC], f32, isOutput=False)
    y_dram = nc.declare_dram_parameter("y", [RPC, G, D], f32, isOutput=True)

    with tile.TileContext(nc) as tc:
        with (
            tc.tile_pool(name="const", bufs=1) as cpool,
            tc.tile_pool(name="x", bufs=9) as xpool,
            tc.tile_pool(name="tmp", bufs=8) as tpool,
            tc.tile_pool(name="tacc", bufs=6) as tapool,
            tc.tile_pool(name="out", bufs=6) as opool,
            tc.tile_pool(name="psum", bufs=4, space="PSUM") as ppool,
        ):
            wt = cpool.tile([128, 2, 2, 128], f32)
            nc.sync.dma_start(wt[:], wm_dram[:])
            st = cpool.tile([128, RPC], f32)
            nc.sync.dma_start(st[:], sc_dram[:])

            xt = {}

            def load_row(r):
                t = xpool.tile([128, 2, D], f32, tag="xrow")
                nc.sync.dma_start(
                    t[:], x_dram[r % (RPC + 2 * P)].rearrange("(h p) d -> p h d", p=128)
                )
                xt[r] = t

            for r in range(5):
                load_row(r)

            for it in range(RPC * repeats):
                i = it % RPC
                if it > 0:
                    load_row(it + 4)
                tacc = []
                for h in range(2):
                    a0 = xt[it][:, h, :]
                    a1 = xt[it + 1][:, h, :]
                    a2 = xt[it + 2][:, h, :]
                    a3 = xt[it + 3][:, h, :]
                    a4 = xt[it + 4][:, h, :]
                    # engines split: gpsimd takes one TT add, vector the rest
                    t1 = tpool.tile([128, D], f32, tag="t1")
                    nc.vector.tensor_tensor(t1[:], a1, a3, add)
                    t2 = tpool.tile([128, D], f32, tag="t2")
                    nc.gpsimd.tensor_tensor(t2[:], a0, a4, add)
                    t3 = tpool.tile([128, D], f32, tag="t3")
                    nc.vector.scalar_tensor_tensor(t3[:], t2[:], w2 / w1, t1[:], mult, add)
                    tt = tapool.tile([128, D], f32, tag="tacc")
                    nc.vector.scalar_tensor_tensor(tt[:], t3[:], w1, a2, mult, add)
                    tacc.append(tt)
                del t1, t2, t3, tt
                for hm in range(2):
                    ps = ppool.tile([128, D], f32, tag="ps")
                    nc.tensor.matmul(
                        ps[:], wt[:, 0, hm, :], tacc[0][:], start=True, stop=False
                    )
                    nc.tensor.matmul(
                        ps[:], wt[:, 1, hm, :], tacc[1][:], start=False, stop=True
                    )
                    ob = opool.tile([128, D], f32, tag="ob")
                    nc.scalar.mul(ob[:], ps[:], st[:, i:i + 1])
                    nc.sync.dma_start(
                        y_dram[i, 128 * hm:128 * hm + 128, :], ob[:]
                    )
    nc.finalize()
    return nc


def _get_program():
    if "nc" not in _cache:
        _cache["nc"] = _build_nc()
        _cache["consts"] = _host_consts()
    return _cache["nc"], _cache["consts"]


def kernel(H, xy=None):
    from concourse.bass_utils import run_bass_kernel_spmd

    nc, (wmat, scales) = _get_program()
    H3 = np.ascontiguousarray(H.reshape(G, G, D).astype(np.float32))
    Hp = np.zeros((G + 2 * P, G, D), dtype=np.float32)
    Hp[P:P + G] = H3
    in_maps = []
    for c in range(NC):
        shard = np.ascontiguousarray(Hp[RPC * c: RPC * c + RPC + 2 * P])
        in_maps.append({"x": shard, "wmat": wmat, "scale": scales[c]})
    res = run_bass_kernel_spmd(nc, in_maps, list(range(NC))).results
    out = np.concatenate([res[c]["y"].reshape(RPC * G, D) for c in range(NC)], axis=0)
    return out
